# revision 34
# baseline (speedup 1.0000x reference)
"""Trainium2 Bass kernel for nn_Attention (B=16, L=1024, E=768), 8 NeuronCores.

reference:
    x = inputs + pe;  q = qustions + pe              # pe: sinusoidal positional enc
    S = x @ q^T / sqrt(E)  per batch item            # [B, L, L]
    p = softmax(S, axis=-1)                          # [B, L, L]
    out = p @ x                                      # [B, L, E]
    returns (out, p)

Sharding: data-parallel over batch. Each of the 8 cores gets 2 batch items.
No cross-core communication.

Device algorithm per item (L=1024 rows processed in 8 chunks of 128):
  - host supplies inputs/qustions pre-TRANSPOSED ([E, L], "d-major") so the
    contraction over E in S = x q^T maps directly onto the TensorEngine
    (contraction is over the partition dim) with no on-device transpose.
    The positional-encoding add happens on device (DVE), in both layouts,
    writing float32r (TF32) tiles as required by full-rate fp32r matmuls.
  - S row-chunk [128, 1024] accumulated in PSUM over 6 K-chunks.
  - softmax without max-subtraction (scores are ~N(0,2); |S/sqrt(E)| < 15,
    far from f32 exp overflow): exp on ACT with fused scale, output in
    bf16, row-sum fused via accum_out. Normalization by 1/sum is folded
    into the two ACT Copy(scale=r) epilogues (p_f32 for DMA, out-chunk
    scale), so transposes can consume exp output immediately.
  - p chunk (bf16) is PE-transposed (128x128 tiles via bf16 identity
    matmul, 1 cyc/row); out chunk = pT.T @ x_natural accumulated over 8
    m-chunks in bf16 (affects only `out`, not `p`).
"""

import math

import numpy as np

B = 16
L = 1024
E = 768
N_CORES = 8
PER_CORE = B // N_CORES  # 2
KC = E // 128  # 6 contraction chunks
LC = L // 128  # 8 row chunks
MC = L // 128  # 8 col chunks
SCALE = 1.0 / math.sqrt(float(E))

_CACHE = {}


def _positional_encoding():
    # identical math to the reference (numpy float32)
    max_len, d_model = 1600, E
    position = np.arange(max_len, dtype=np.float32)[:, None]
    div_term = np.exp(
        np.arange(0, d_model, 2, dtype=np.float32) * (-math.log(10000.0) / d_model)
    )
    pe = np.zeros((max_len, d_model), dtype=np.float32)
    pe[:, 0::2] = np.sin(position * div_term)
    pe[:, 1::2] = np.cos(position * div_term)
    return pe[:L]  # [L, E]


def build_nc():
    import concourse.mybir as mybir
    import concourse.tile as tile
    from concourse import bacc
    from concourse.masks import make_identity
    from contextlib import ExitStack

    f32 = mybir.dt.float32
    f32r = mybir.dt.float32r
    bf16 = mybir.dt.bfloat16
    f16 = mybir.dt.float16
    Exp = mybir.ActivationFunctionType.Exp
    Copy = mybir.ActivationFunctionType.Copy

    nc = bacc.Bacc()

    # in_t/q_t/pe_t ship as f16: for unit-scale data f16 (e5m10) carries the
    # same ~2^-11 relative error as TF32/fp32r matmuls, at half the DMA bytes
    # and full TensorEngine rate (1 cyc/row).
    in_t = nc.declare_dram_parameter("in_t", [PER_CORE, E, L], f16, isOutput=False)
    q_t = nc.declare_dram_parameter("q_t", [PER_CORE, E, L], f16, isOutput=False)
    in_n = nc.declare_dram_parameter("in_n", [PER_CORE, L, E], bf16, isOutput=False)
    # pe_t ships as f16: |pe| <= 1 so f16 quantization (~2.4e-4 abs) is at
    # the TF32 rounding level, and it halves the cold-start critical bytes.
    pe_t = nc.declare_dram_parameter("pe_t", [E, L], f16, isOutput=False)
    pe_n = nc.declare_dram_parameter("pe_n", [L, E], bf16, isOutput=False)
    out = nc.declare_dram_parameter("out", [PER_CORE, L, E], f32, isOutput=True)
    p = nc.declare_dram_parameter("p", [PER_CORE, L, L], f32, isOutput=True)

    with ExitStack() as ctx:
        tc = ctx.enter_context(tile.TileContext(nc))
        const_pool = ctx.enter_context(tc.tile_pool(name="const", bufs=1))
        item_pool = ctx.enter_context(tc.tile_pool(name="item", bufs=2))
        pbf_pool = ctx.enter_context(tc.tile_pool(name="pbf", bufs=4))
        pf_pool = ctx.enter_context(tc.tile_pool(name="pf", bufs=2))
        pt_pool = ctx.enter_context(tc.tile_pool(name="pt", bufs=3))
        osb_pool = ctx.enter_context(tc.tile_pool(name="osb", bufs=2))
        stat_pool = ctx.enter_context(tc.tile_pool(name="stat", bufs=8))
        s_psum = ctx.enter_context(tc.tile_pool(name="s_psum", bufs=2, space="PSUM"))
        t_psum = ctx.enter_context(tc.tile_pool(name="t_psum", bufs=2, space="PSUM"))
        o_psum = ctx.enter_context(tc.tile_pool(name="o_psum", bufs=2, space="PSUM"))

        ident = const_pool.tile([128, 128], bf16)
        make_identity(nc, ident)

        # PE clock warm-up: the HAM clock gate keeps the TensorEngine at
        # half rate until it has been busy for ~4us. The first ~25us of the
        # kernel are input-DMA-bound with no real matmul work, so the clock
        # would ramp only ~30us in (and every early matmul would run 2x
        # slow). Issue dependency-free dummy matmuls (identity x zeros into
        # a scratch PSUM tile) that keep the PE array busy until real data
        # arrives, at which point it runs at full 2.4 GHz immediately.
        warm_src = const_pool.tile([128, 384], bf16)
        nc.vector.memset(warm_src, 0.0)
        def pe_filler(n, tgt):
            for w in range(n):
                nc.tensor.matmul(
                    tgt, lhsT=ident, rhs=warm_src, start=True, stop=True
                )

        warm_ps = o_psum.tile([128, 384], f32, tag="ops", name="warm_ps")
        pe_filler(15, warm_ps)

        peT = [None] * KC
        peN = [None] * MC

        # ---- item-0 load phase. matmul1 of the first row chunk needs
        # (pe + q + x transposed) fully; x_natural is only needed when the
        # first chunk reaches matmul2, so it loads last. Item 1's loads are
        # emitted interleaved into item 0's chunk loop below, so its DVE
        # pe-adds slot into DVE-idle windows between chunk epilogues instead
        # of head-of-line-blocking them while item 1's DMAs stream in.
        def load_qx(it, k, warm):
            tq = item_pool.tile([128, L], f16, name=f"qT{it}_{k}", tag=f"qT{k}")
            nc.gpsimd.dma_start(out=tq, in_=q_t[it, k * 128:(k + 1) * 128, :])
            nc.vector.tensor_add(tq, tq, peT[k])
            tx = item_pool.tile([128, L], f16, name=f"xT{it}_{k}", tag=f"xT{k}")
            nc.gpsimd.dma_start(out=tx, in_=in_t[it, k * 128:(k + 1) * 128, :])
            nc.vector.tensor_add(tx, tx, peT[k])
            if warm:
                # Data-paced PE warm-up: these matmuls consume the chunk
                # that just landed, so they pace the PE queue with the
                # DMA stream and keep the HAM clock at 8/8 until the
                # real matmuls have data (no fixed-count guesswork).
                nc.tensor.matmul(
                    warm_ps, lhsT=tx[:, 0:128], rhs=tq[:, 0:384],
                    start=True, stop=True,
                )
                nc.tensor.matmul(
                    warm_ps, lhsT=tx[:, 0:128], rhs=tq[:, 384:768],
                    start=True, stop=True,
                )
                nc.tensor.matmul(
                    warm_ps, lhsT=tq[:, 0:128], rhs=tx[:, 0:384],
                    start=True, stop=True,
                )
                nc.tensor.matmul(
                    warm_ps, lhsT=tq[:, 0:128], rhs=tx[:, 384:768],
                    start=True, stop=True,
                )
            return tq, tx

        def load_xn(it, k):
            t = item_pool.tile([128, E], bf16, name=f"xN{it}_{k}", tag=f"xN{k}")
            nc.gpsimd.dma_start(out=t, in_=in_n[it, k * 128:(k + 1) * 128, :])
            nc.vector.tensor_add(t, t, peN[k])
            return t

        xTs = [[], []]
        qTs = [[], []]
        xNs = [[], []]
        for k in range(KC):
            t = const_pool.tile([128, L], f16, name=f"peT{k}", tag=f"peT{k}")
            nc.gpsimd.dma_start(out=t, in_=pe_t[k * 128:(k + 1) * 128, :])
            peT[k] = t
            tq, tx = load_qx(0, k, warm=True)
            qTs[0].append(tq)
            xTs[0].append(tx)
        for k in range(MC):
            t = const_pool.tile([128, E], bf16, name=f"peN{k}", tag=f"peN{k}")
            nc.gpsimd.dma_start(out=t, in_=pe_n[k * 128:(k + 1) * 128, :])
            peN[k] = t
            xNs[0].append(load_xn(0, k))

        for it in range(PER_CORE):
            xT, qT, xN = xTs[it], qTs[it], xNs[it]
            for j in range(LC):
                if it == 0 and PER_CORE > 1:
                    # stream item 1's loads between item 0's chunks
                    if j < KC:
                        tq, tx = load_qx(1, j, warm=False)
                        qTs[1].append(tq)
                        xTs[1].append(tx)
                    elif j == KC:
                        for k in range(0, 4):
                            xNs[1].append(load_xn(1, k))
                    else:
                        for k in range(4, MC):
                            xNs[1].append(load_xn(1, k))
                # ---- S = x @ q^T (row chunk j), fp32r full-rate ----
                S = s_psum.tile([128, L], f32, tag="S")
                for k in range(KC):
                    lhs = xT[k][:, j * 128:(j + 1) * 128]
                    nc.tensor.matmul(
                        S[:, 0:512], lhsT=lhs, rhs=qT[k][:, 0:512],
                        start=(k == 0), stop=(k == KC - 1),
                    )
                    nc.tensor.matmul(
                        S[:, 512:1024], lhsT=lhs, rhs=qT[k][:, 512:1024],
                        start=(k == 0), stop=(k == KC - 1),
                    )

                if it == PER_CORE - 1 and j >= LC - 2:
                    # Tail filler: keep the PE clock at 8/8 through the last
                    # chunks' softmax latency so their transposes+matmul2
                    # run at full rate during the pipeline drain.
                    tail_ps = s_psum.tile(
                        [128, L], f32, tag="S", name=f"tail_ps{j}"
                    )
                    pe_filler(16 if j == LC - 2 else 40, tail_ps[:, 0:384])

                # ---- softmax (no max subtraction; |S*c| < 15) ----
                p_bf = pbf_pool.tile([128, L], bf16, tag="p_bf")
                nc.scalar.activation(p_bf[:, 0:512], S[:, 0:512], Exp, scale=SCALE)
                nc.scalar.activation(
                    p_bf[:, 512:1024], S[:, 512:1024], Exp, scale=SCALE
                )
                rsum = stat_pool.tile([128, 1], f32, tag="rsum")
                nc.vector.tensor_reduce(
                    rsum, p_bf, axis=mybir.AxisListType.X, op=mybir.AluOpType.add
                )
                rcp = stat_pool.tile([128, 1], f32, tag="rcp")
                nc.vector.reciprocal(rcp, rsum)

                # p (f32, normalized) for the p_attn output. The last two
                # chunks normalize on DVE (idle at the tail) so the final
                # exp/out-scale chain on ACT drains sooner.
                p_f32 = pf_pool.tile([128, L], f32, tag="p_f32")
                if it == PER_CORE - 1 and j >= LC - 2:
                    nc.vector.tensor_scalar_mul(p_f32, p_bf, rcp)
                else:
                    nc.scalar.activation(
                        p_f32[:, 0:512], p_bf[:, 0:512], Copy, scale=rcp
                    )
                    nc.scalar.activation(
                        p_f32[:, 512:1024], p_bf[:, 512:1024], Copy, scale=rcp
                    )
                nc.sync.dma_start(out=p[it, j * 128:(j + 1) * 128, :], in_=p_f32)

                # ---- pT via PE transpose of the (unnormalized) bf16 exp ----
                pT = pt_pool.tile([128, MC, 128], bf16, tag="pT")
                for h in range(2):
                    tp = t_psum.tile([128, 4, 128], bf16, tag="tp")
                    for kk in range(4):
                        m = h * 4 + kk
                        nc.tensor.transpose(
                            tp[:, kk, :], p_bf[:, m * 128:(m + 1) * 128], ident
                        )
                    nc.vector.tensor_copy(pT[:, h * 4:(h + 1) * 4, :], tp)

                # ---- out chunk = (pT.T @ x_natural) * rcp ----
                o_sb = osb_pool.tile([128, E], f32, tag="osb")
                o_ps0 = o_psum.tile([128, 384], f32, tag="ops")
                o_ps1 = o_psum.tile([128, 384], f32, tag="ops")
                for k in range(MC):
                    nc.tensor.matmul(
                        o_ps0, lhsT=pT[:, k, :], rhs=xN[k][:, 0:384],
                        start=(k == 0), stop=(k == MC - 1),
                    )
                    nc.tensor.matmul(
                        o_ps1, lhsT=pT[:, k, :], rhs=xN[k][:, 384:768],
                        start=(k == 0), stop=(k == MC - 1),
                    )
                nc.scalar.activation(o_sb[:, 0:384], o_ps0, Copy, scale=rcp)
                nc.scalar.activation(o_sb[:, 384:768], o_ps1, Copy, scale=rcp)
                nc.sync.dma_start(out=out[it, j * 128:(j + 1) * 128, :], in_=o_sb)

    # Bacc.finalize runs generate_event_semaphores, which splits multi-sem
    # waits (HW allows 1 wait per instruction) into EventSemaphore insts.
    nc.finalize()
    return nc


def _get_nc():
    if "nc" not in _CACHE:
        _CACHE["nc"] = build_nc()
    return _CACHE["nc"]


def _prep_in_maps(inputs, qustions):
    import ml_dtypes

    inputs = np.asarray(inputs, dtype=np.float32)
    qustions = np.asarray(qustions, dtype=np.float32)
    pe = _positional_encoding()  # [L, E] f32

    in_t = np.ascontiguousarray(inputs.transpose(0, 2, 1)).astype(np.float16)  # [B, E, L]
    q_t = np.ascontiguousarray(qustions.transpose(0, 2, 1)).astype(np.float16)
    in_n = inputs.astype(ml_dtypes.bfloat16)
    pe_t = np.ascontiguousarray(pe.T).astype(np.float16)  # [E, L]
    pe_n = pe.astype(ml_dtypes.bfloat16)

    in_maps = []
    for c in range(N_CORES):
        s = slice(c * PER_CORE, (c + 1) * PER_CORE)
        in_maps.append(
            {
                "in_t": np.ascontiguousarray(in_t[s]),
                "q_t": np.ascontiguousarray(q_t[s]),
                "in_n": np.ascontiguousarray(in_n[s]),
                "pe_t": pe_t,
                "pe_n": pe_n,
            }
        )
    return in_maps


def run(inputs, qustions, trace=False, tmpdir=None):
    """Build+compile (cached), execute on 8 NeuronCores, gather. Returns
    (out, p_attn, BassKernelResults)."""
    from concourse.bass_utils import run_bass_kernel_spmd

    nc = _get_nc()
    in_maps = _prep_in_maps(inputs, qustions)
    res = run_bass_kernel_spmd(
        nc, in_maps, core_ids=list(range(N_CORES)), trace=trace, tmpdir=tmpdir
    )
    out = np.concatenate([r["out"] for r in res.results], axis=0)
    p_attn = np.concatenate([r["p"] for r in res.results], axis=0)
    return out, p_attn, res


def kernel(inputs, qustions):
    out, p_attn, _ = run(inputs, qustions)
    return out, p_attn


# revision 35
# speedup vs baseline: 1.0634x; 1.0634x over previous
"""Trainium2 Bass kernel for nn_Attention (B=16, L=1024, E=768), 8 NeuronCores.

reference:
    x = inputs + pe;  q = qustions + pe              # pe: sinusoidal positional enc
    S = x @ q^T / sqrt(E)  per batch item            # [B, L, L]
    p = softmax(S, axis=-1)                          # [B, L, L]
    out = p @ x                                      # [B, L, E]
    returns (out, p)

Sharding: data-parallel over batch. Each of the 8 cores gets 2 batch items.
No cross-core communication.

Device algorithm per item (L=1024 rows processed in 8 chunks of 128):
  - host supplies inputs/qustions pre-TRANSPOSED ([E, L], "d-major") so the
    contraction over E in S = x q^T maps directly onto the TensorEngine
    (contraction is over the partition dim) with no on-device transpose.
    The positional-encoding add happens on device (DVE), in both layouts,
    writing float32r (TF32) tiles as required by full-rate fp32r matmuls.
  - S row-chunk [128, 1024] accumulated in PSUM over 6 K-chunks.
  - softmax without max-subtraction (scores are ~N(0,2); |S/sqrt(E)| < 15,
    far from f32 exp overflow): exp on ACT with fused scale, output in
    bf16, row-sum fused via accum_out. Normalization by 1/sum is folded
    into the two ACT Copy(scale=r) epilogues (p_f32 for DMA, out-chunk
    scale), so transposes can consume exp output immediately.
  - p chunk (bf16) is PE-transposed (128x128 tiles via bf16 identity
    matmul, 1 cyc/row); out chunk = pT.T @ x_natural accumulated over 8
    m-chunks in bf16 (affects only `out`, not `p`).
"""

import math

import numpy as np

B = 16
L = 1024
E = 768
N_CORES = 8
PER_CORE = B // N_CORES  # 2
KC = E // 128  # 6 contraction chunks
LC = L // 128  # 8 row chunks
MC = L // 128  # 8 col chunks
SCALE = 1.0 / math.sqrt(float(E))

_CACHE = {}


def _positional_encoding():
    # identical math to the reference (numpy float32)
    max_len, d_model = 1600, E
    position = np.arange(max_len, dtype=np.float32)[:, None]
    div_term = np.exp(
        np.arange(0, d_model, 2, dtype=np.float32) * (-math.log(10000.0) / d_model)
    )
    pe = np.zeros((max_len, d_model), dtype=np.float32)
    pe[:, 0::2] = np.sin(position * div_term)
    pe[:, 1::2] = np.cos(position * div_term)
    return pe[:L]  # [L, E]


def build_nc():
    import concourse.mybir as mybir
    import concourse.tile as tile
    from concourse import bacc
    from concourse.masks import make_identity
    from contextlib import ExitStack

    f32 = mybir.dt.float32
    f32r = mybir.dt.float32r
    bf16 = mybir.dt.bfloat16
    f16 = mybir.dt.float16
    Exp = mybir.ActivationFunctionType.Exp
    Copy = mybir.ActivationFunctionType.Copy

    nc = bacc.Bacc()

    # in_t/q_t/pe_t ship as f16: for unit-scale data f16 (e5m10) carries the
    # same ~2^-11 relative error as TF32/fp32r matmuls, at half the DMA bytes
    # and full TensorEngine rate (1 cyc/row).
    in_t = nc.declare_dram_parameter("in_t", [PER_CORE, E, L], f16, isOutput=False)
    q_t = nc.declare_dram_parameter("q_t", [PER_CORE, E, L], f16, isOutput=False)
    in_n = nc.declare_dram_parameter("in_n", [PER_CORE, L, E], bf16, isOutput=False)
    # pe_t ships as f16: |pe| <= 1 so f16 quantization (~2.4e-4 abs) is at
    # the TF32 rounding level, and it halves the cold-start critical bytes.
    pe_t = nc.declare_dram_parameter("pe_t", [E, L], f16, isOutput=False)
    pe_n = nc.declare_dram_parameter("pe_n", [L, E], bf16, isOutput=False)
    out = nc.declare_dram_parameter("out", [PER_CORE, L, E], f32, isOutput=True)
    p = nc.declare_dram_parameter("p", [PER_CORE, L, L], f32, isOutput=True)

    with ExitStack() as ctx:
        tc = ctx.enter_context(tile.TileContext(nc))
        const_pool = ctx.enter_context(tc.tile_pool(name="const", bufs=1))
        item_pool = ctx.enter_context(tc.tile_pool(name="item", bufs=2))
        pbf_pool = ctx.enter_context(tc.tile_pool(name="pbf", bufs=4))
        pf_pool = ctx.enter_context(tc.tile_pool(name="pf", bufs=2))
        pt_pool = ctx.enter_context(tc.tile_pool(name="pt", bufs=3))
        osb_pool = ctx.enter_context(tc.tile_pool(name="osb", bufs=2))
        stat_pool = ctx.enter_context(tc.tile_pool(name="stat", bufs=8))
        s_psum = ctx.enter_context(tc.tile_pool(name="s_psum", bufs=2, space="PSUM"))
        t_psum = ctx.enter_context(tc.tile_pool(name="t_psum", bufs=2, space="PSUM"))
        o_psum = ctx.enter_context(tc.tile_pool(name="o_psum", bufs=2, space="PSUM"))

        ident = const_pool.tile([128, 128], bf16)
        make_identity(nc, ident)

        # PE clock warm-up: the HAM clock gate keeps the TensorEngine at
        # half rate until it has been busy for ~4us. The first ~25us of the
        # kernel are input-DMA-bound with no real matmul work, so the clock
        # would ramp only ~30us in (and every early matmul would run 2x
        # slow). Issue dependency-free dummy matmuls (identity x zeros into
        # a scratch PSUM tile) that keep the PE array busy until real data
        # arrives, at which point it runs at full 2.4 GHz immediately.
        warm_src = const_pool.tile([128, 384], bf16)
        nc.vector.memset(warm_src, 0.0)
        def pe_filler(n, tgt):
            for w in range(n):
                nc.tensor.matmul(
                    tgt, lhsT=ident, rhs=warm_src, start=True, stop=True
                )

        warm_ps = o_psum.tile([128, 384], f32, tag="ops", name="warm_ps")
        pe_filler(15, warm_ps)

        peT = [None] * KC
        peN = [None] * MC

        # ---- item-0 load phase. matmul1 of the first row chunk needs
        # (pe + q + x transposed) fully; x_natural is only needed when the
        # first chunk reaches matmul2, so it loads last. Item 1's loads are
        # emitted interleaved into item 0's chunk loop below, so its DVE
        # pe-adds slot into DVE-idle windows between chunk epilogues instead
        # of head-of-line-blocking them while item 1's DMAs stream in.
        def load_qx(it, k, warm):
            tq = item_pool.tile([128, L], f16, name=f"qT{it}_{k}", tag=f"qT{k}")
            nc.gpsimd.dma_start(out=tq, in_=q_t[it, k * 128:(k + 1) * 128, :])
            nc.vector.tensor_add(tq, tq, peT[k])
            tx = item_pool.tile([128, L], f16, name=f"xT{it}_{k}", tag=f"xT{k}")
            nc.gpsimd.dma_start(out=tx, in_=in_t[it, k * 128:(k + 1) * 128, :])
            nc.vector.tensor_add(tx, tx, peT[k])
            if warm:
                # Data-paced PE warm-up: these matmuls consume the chunk
                # that just landed, so they pace the PE queue with the
                # DMA stream and keep the HAM clock at 8/8 until the
                # real matmuls have data (no fixed-count guesswork).
                nc.tensor.matmul(
                    warm_ps, lhsT=tx[:, 0:128], rhs=tq[:, 0:384],
                    start=True, stop=True,
                )
                nc.tensor.matmul(
                    warm_ps, lhsT=tx[:, 0:128], rhs=tq[:, 384:768],
                    start=True, stop=True,
                )
                nc.tensor.matmul(
                    warm_ps, lhsT=tq[:, 0:128], rhs=tx[:, 0:384],
                    start=True, stop=True,
                )
                nc.tensor.matmul(
                    warm_ps, lhsT=tq[:, 0:128], rhs=tx[:, 384:768],
                    start=True, stop=True,
                )
            return tq, tx

        def load_xn(it, k):
            t = item_pool.tile([128, E], bf16, name=f"xN{it}_{k}", tag=f"xN{k}")
            nc.gpsimd.dma_start(out=t, in_=in_n[it, k * 128:(k + 1) * 128, :])
            nc.vector.tensor_add(t, t, peN[k])
            return t

        xTs = [[], []]
        qTs = [[], []]
        xNs = [[], []]
        for k in range(KC):
            t = const_pool.tile([128, L], f16, name=f"peT{k}", tag=f"peT{k}")
            nc.gpsimd.dma_start(out=t, in_=pe_t[k * 128:(k + 1) * 128, :])
            peT[k] = t
            tq, tx = load_qx(0, k, warm=True)
            qTs[0].append(tq)
            xTs[0].append(tx)
        for k in range(MC):
            t = const_pool.tile([128, E], bf16, name=f"peN{k}", tag=f"peN{k}")
            nc.gpsimd.dma_start(out=t, in_=pe_n[k * 128:(k + 1) * 128, :])
            peN[k] = t
            xNs[0].append(load_xn(0, k))

        for it in range(PER_CORE):
            xT, qT, xN = xTs[it], qTs[it], xNs[it]
            for j in range(LC):
                if it == 0 and PER_CORE > 1:
                    # stream item 1's loads between item 0's chunks
                    if j < KC:
                        tq, tx = load_qx(1, j, warm=False)
                        qTs[1].append(tq)
                        xTs[1].append(tx)
                    elif j == KC:
                        for k in range(0, 4):
                            xNs[1].append(load_xn(1, k))
                    else:
                        for k in range(4, MC):
                            xNs[1].append(load_xn(1, k))
                # ---- S = x @ q^T (row chunk j), fp32r full-rate ----
                S = s_psum.tile([128, L], f32, tag="S")
                for k in range(KC):
                    lhs = xT[k][:, j * 128:(j + 1) * 128]
                    nc.tensor.matmul(
                        S[:, 0:512], lhsT=lhs, rhs=qT[k][:, 0:512],
                        start=(k == 0), stop=(k == KC - 1),
                    )
                    nc.tensor.matmul(
                        S[:, 512:1024], lhsT=lhs, rhs=qT[k][:, 512:1024],
                        start=(k == 0), stop=(k == KC - 1),
                    )

                if it == PER_CORE - 1 and j == LC - 1:
                    # Tail filler: keep the PE clock at 8/8 through the last
                    # chunk's softmax latency so its transposes+matmul2 run
                    # at full rate during the pipeline drain. Sized to the
                    # exp latency only — more would delay the transposes.
                    tail_ps = s_psum.tile(
                        [128, L], f32, tag="S", name=f"tail_ps{j}"
                    )
                    pe_filler(12, tail_ps[:, 0:384])

                # ---- softmax (no max subtraction; |S*c| < 15) ----
                p_bf = pbf_pool.tile([128, L], bf16, tag="p_bf")
                nc.scalar.activation(p_bf[:, 0:512], S[:, 0:512], Exp, scale=SCALE)
                nc.scalar.activation(
                    p_bf[:, 512:1024], S[:, 512:1024], Exp, scale=SCALE
                )
                rsum = stat_pool.tile([128, 1], f32, tag="rsum")
                nc.vector.tensor_reduce(
                    rsum, p_bf, axis=mybir.AxisListType.X, op=mybir.AluOpType.add
                )
                rcp = stat_pool.tile([128, 1], f32, tag="rcp")
                nc.vector.reciprocal(rcp, rsum)

                # p (f32, normalized) for the p_attn output. The last two
                # chunks normalize on DVE (idle at the tail) so the final
                # exp/out-scale chain on ACT drains sooner.
                p_f32 = pf_pool.tile([128, L], f32, tag="p_f32")
                if it == PER_CORE - 1 and j >= LC - 2:
                    nc.vector.tensor_scalar_mul(p_f32, p_bf, rcp)
                else:
                    nc.scalar.activation(
                        p_f32[:, 0:512], p_bf[:, 0:512], Copy, scale=rcp
                    )
                    nc.scalar.activation(
                        p_f32[:, 512:1024], p_bf[:, 512:1024], Copy, scale=rcp
                    )
                nc.sync.dma_start(out=p[it, j * 128:(j + 1) * 128, :], in_=p_f32)

                # ---- pT via PE transpose of the (unnormalized) bf16 exp ----
                pT = pt_pool.tile([128, MC, 128], bf16, tag="pT")
                for h in range(2):
                    tp = t_psum.tile([128, 4, 128], bf16, tag="tp")
                    for kk in range(4):
                        m = h * 4 + kk
                        nc.tensor.transpose(
                            tp[:, kk, :], p_bf[:, m * 128:(m + 1) * 128], ident
                        )
                    nc.vector.tensor_copy(pT[:, h * 4:(h + 1) * 4, :], tp)

                # ---- out chunk = (pT.T @ x_natural) * rcp ----
                o_sb = osb_pool.tile([128, E], f32, tag="osb")
                o_ps0 = o_psum.tile([128, 384], f32, tag="ops")
                o_ps1 = o_psum.tile([128, 384], f32, tag="ops")
                for k in range(MC):
                    nc.tensor.matmul(
                        o_ps0, lhsT=pT[:, k, :], rhs=xN[k][:, 0:384],
                        start=(k == 0), stop=(k == MC - 1),
                    )
                    nc.tensor.matmul(
                        o_ps1, lhsT=pT[:, k, :], rhs=xN[k][:, 384:768],
                        start=(k == 0), stop=(k == MC - 1),
                    )
                nc.scalar.activation(o_sb[:, 0:384], o_ps0, Copy, scale=rcp)
                nc.scalar.activation(o_sb[:, 384:768], o_ps1, Copy, scale=rcp)
                nc.sync.dma_start(out=out[it, j * 128:(j + 1) * 128, :], in_=o_sb)

    # Bacc.finalize runs generate_event_semaphores, which splits multi-sem
    # waits (HW allows 1 wait per instruction) into EventSemaphore insts.
    nc.finalize()
    return nc


def _get_nc():
    if "nc" not in _CACHE:
        _CACHE["nc"] = build_nc()
    return _CACHE["nc"]


def _prep_in_maps(inputs, qustions):
    import ml_dtypes

    inputs = np.asarray(inputs, dtype=np.float32)
    qustions = np.asarray(qustions, dtype=np.float32)
    pe = _positional_encoding()  # [L, E] f32

    in_t = np.ascontiguousarray(inputs.transpose(0, 2, 1)).astype(np.float16)  # [B, E, L]
    q_t = np.ascontiguousarray(qustions.transpose(0, 2, 1)).astype(np.float16)
    in_n = inputs.astype(ml_dtypes.bfloat16)
    pe_t = np.ascontiguousarray(pe.T).astype(np.float16)  # [E, L]
    pe_n = pe.astype(ml_dtypes.bfloat16)

    in_maps = []
    for c in range(N_CORES):
        s = slice(c * PER_CORE, (c + 1) * PER_CORE)
        in_maps.append(
            {
                "in_t": np.ascontiguousarray(in_t[s]),
                "q_t": np.ascontiguousarray(q_t[s]),
                "in_n": np.ascontiguousarray(in_n[s]),
                "pe_t": pe_t,
                "pe_n": pe_n,
            }
        )
    return in_maps


def run(inputs, qustions, trace=False, tmpdir=None):
    """Build+compile (cached), execute on 8 NeuronCores, gather. Returns
    (out, p_attn, BassKernelResults)."""
    from concourse.bass_utils import run_bass_kernel_spmd

    nc = _get_nc()
    in_maps = _prep_in_maps(inputs, qustions)
    res = run_bass_kernel_spmd(
        nc, in_maps, core_ids=list(range(N_CORES)), trace=trace, tmpdir=tmpdir
    )
    out = np.concatenate([r["out"] for r in res.results], axis=0)
    p_attn = np.concatenate([r["p"] for r in res.results], axis=0)
    return out, p_attn, res


def kernel(inputs, qustions):
    out, p_attn, _ = run(inputs, qustions)
    return out, p_attn
